# revision 25
# baseline (speedup 1.0000x reference)
"""Multi-head attention block (QKV proj + softmax attention + out proj) on 8
Trainium2 NeuronCores.

Sharding: core c handles batch b = c//2 and head-half hh = c%2 (8 of the 16
heads).  Each core computes its QKV column slice, full attention for its 8
heads, and a partial output projection (contracting only its heads' dims).
Host sums the two partials per batch and adds b_proj.

Device layouts (all bf16 storage, fp32 PSUM accumulation):
  xT  [1024 d, 2048 tok]      (x[b] transposed on host)
  Q^T/K^T [512 col, 2048 tok] as 4 tiles [128, 2048]  (pair pr = heads 2pr,2pr+1)
  V   [2048 tok, 8 x (64 dims + 1 ones col)] as 16 tiles [128, 520]
  scores^T tiles [128 kj, 1024 qi] in PSUM -> exp on ScalarE -> SBUF bf16
  attn@V per head as out^T [65 (64 d + denom), 512 qi] accumulated in PSUM;
  the ones column in V makes PSUM row 64 the softmax denominator for free
  (matmul cost depends only on the moving free size).
"""

import os

import numpy as np

B = 4
S = 2048
D = 1024
NUM_HEADS = 16
HD = 64
SCALE = HD**-0.5
# bf16 Schraudolph exp constants: bits16 = round(x*SCALE*128/ln2 + B)
import math
EXPA = SCALE * 128.0 / math.log(2.0)
EXPB = 127.0 * 128.0 - 5.5
N_CORES = 8
P = 128
NPAIR = 4  # head pairs per core
DB = D // P  # 8 contraction blocks

_CACHE = {}


def _build_nc(repeat=1, no_exp=False):
    import concourse.bacc as bacc
    import concourse.mybir as mybir
    import concourse.tile as tile

    bf16 = mybir.dt.bfloat16
    f16 = mybir.dt.float16
    f32 = mybir.dt.float32
    AF = mybir.ActivationFunctionType
    ALU = mybir.AluOpType

    nc = bacc.Bacc("TRN2", target_bir_lowering=False, debug=False,
                   num_devices=N_CORES)

    xt_d = nc.dram_tensor("xt", [D, S], bf16, kind="ExternalInput")
    wq_d = nc.dram_tensor("wq", [D, 512], bf16, kind="ExternalInput")
    wk_d = nc.dram_tensor("wk", [D, 512], bf16, kind="ExternalInput")
    wv_d = nc.dram_tensor("wv", [D, 512], bf16, kind="ExternalInput")
    wp_d = nc.dram_tensor("wp", [512, D], bf16, kind="ExternalInput")
    y_d = nc.dram_tensor("y", [S, D], f32, kind="ExternalOutput")

    with tile.TileContext(nc) as tc:
        from contextlib import ExitStack, nullcontext

        with ExitStack() as ctx:
            const_pool = ctx.enter_context(tc.tile_pool(name="const", bufs=1))
            w_pool = ctx.enter_context(tc.tile_pool(name="w", bufs=1))
            wp_pool = ctx.enter_context(tc.tile_pool(name="wp", bufs=1))
            qt_pool = ctx.enter_context(tc.tile_pool(name="qt", bufs=1))
            kt_pool = ctx.enter_context(tc.tile_pool(name="kt", bufs=1))
            v_pool = ctx.enter_context(tc.tile_pool(name="v", bufs=1))
            ot_pool = ctx.enter_context(tc.tile_pool(name="ot", bufs=1))
            xt_pool = ctx.enter_context(tc.tile_pool(name="xt", bufs=1))
            att_sb = ctx.enter_context(tc.tile_pool(name="att_sb", bufs=2))
            y_pool = ctx.enter_context(tc.tile_pool(name="y", bufs=3))
            # PSUM banks: s 2x2 + qk 1 + outa 1 + outb 1 + rb 1 = 8
            psp = ctx.enter_context(tc.tile_pool(name="ps", bufs=1,
                                                 space="PSUM"))

            ones64 = const_pool.tile([P, HD], bf16, tag="ones64",
                                     name="ones64")
            nc.vector.memset(ones64[:], 1.0)
            e_const = None
            if no_exp:
                e_const = const_pool.tile([P, 1024], bf16, tag="e_const",
                                          name="e_const")
                nc.vector.memset(e_const[:], 0.001)

            wq_t = [w_pool.tile([P, 512], bf16, name=f"wq{i}")
                    for i in range(DB)]
            wk_t = [w_pool.tile([P, 512], bf16, name=f"wk{i}")
                    for i in range(DB)]
            wv_t = [w_pool.tile([P, 512], bf16, name=f"wv{i}")
                    for i in range(DB)]
            for i in range(DB):
                nc.sync.dma_start(wq_t[i][:], wq_d[i * P:(i + 1) * P, :])
                nc.sync.dma_start(wk_t[i][:], wk_d[i * P:(i + 1) * P, :])
                nc.sync.dma_start(wv_t[i][:], wv_d[i * P:(i + 1) * P, :])
            wp_t = {}
            for pr in range(NPAIR):
                for do in range(2):
                    t = wp_pool.tile([P, 512], bf16, name=f"wp{pr}_{do}")
                    nc.sync.dma_start(
                        t[:], wp_d[pr * P:(pr + 1) * P, do * 512:(do + 1) * 512])
                    wp_t[(pr, do)] = t

            qt_t = [qt_pool.tile([P, S], bf16, name=f"qt{p}")
                    for p in range(NPAIR)]
            kt_t = [kt_pool.tile([P, S], bf16, name=f"kt{p}")
                    for p in range(NPAIR)]
            # 8 head blocks of 65 cols: 64 V dims + a ones column whose AV
            # matmul output row is the softmax denominator.
            v_t = [v_pool.tile([P, 8 * 65], bf16, name=f"v{i}")
                   for i in range(16)]
            for i in range(16):
                nc.vector.memset(v_t[i][:], 1.0)
            ot_t = {}
            for pr in range(NPAIR):
                for qh in range(2):
                    ot_t[(pr, qh)] = ot_pool.tile([P, 1024], bf16,
                                                  name=f"ot{pr}_{qh}")

            loop_cm = (tc.For_i(0, repeat, 1) if repeat > 1 else nullcontext())
            with loop_cm:
                xt_t = [xt_pool.tile([P, S], bf16, tag=f"xt{i}",
                                     name=f"xt{i}") for i in range(DB)]
                for i in range(DB):
                    # xt prefetch rides the Activation HWDGE queue (its SEQ
                    # is idle at body start) so the SP queue's y writes from
                    # the previous iteration can't delay it
                    nc.scalar.dma_start(xt_t[i][:],
                                        xt_d[i * P:(i + 1) * P, :])

                def emit_qk_group(pr, ch, w_t, dst, tag="qk"):
                    co = ch * 512
                    ps = psp.tile([P, 512], f32, tag=tag, name="qk_ps")
                    for db in range(DB):
                        nc.tensor.matmul(
                            ps[:],
                            lhsT=w_t[db][:, pr * P:(pr + 1) * P],
                            rhs=xt_t[db][:, co:co + 512],
                            start=(db == 0), stop=(db == DB - 1))
                    nc.vector.tensor_copy(dst[:, co:co + 512], ps[:])

                def emit_v_group(ti, tag):
                    ps = psp.tile([P, 512], f32, tag=tag, name="v_ps")
                    for db in range(DB):
                        nc.tensor.matmul(
                            ps[:],
                            lhsT=xt_t[db][:, ti * P:(ti + 1) * P],
                            rhs=wv_t[db][:],
                            start=(db == 0), stop=(db == DB - 1))
                    dst = v_t[ti].rearrange("p (h c) -> p h c", h=8)[:, :, 0:64]
                    nc.vector.tensor_copy(
                        dst, ps.rearrange("p (h c) -> p h c", h=8))

                def emit_proj_piece(qh, tv, do, tag):
                    ti = qh * 8 + tv
                    off = tv * P
                    yps = psp.tile([P, 512], f32, tag=tag,
                                   bufs=(2 if tag == "s" else 1),
                                   name="y_ps")
                    for pr2 in range(NPAIR):
                        nc.tensor.matmul(
                            yps[:],
                            lhsT=ot_t[(pr2, qh)][:, off:off + P],
                            rhs=wp_t[(pr2, do)][:],
                            start=(pr2 == 0), stop=(pr2 == NPAIR - 1))
                    yt = y_pool.tile([P, 512], f32, tag="ysb", name="ysb")
                    if qh == 1:
                        # tail: ScalarE idles once exp is done; it evacuates
                        # PSUM so DVE (busy with normalize) isn't on the path
                        nc.scalar.activation(yt[:], yps[:], AF.Copy)
                    else:
                        nc.vector.tensor_copy(yt[:], yps[:])
                    nc.sync.dma_start(
                        y_d[ti * P:(ti + 1) * P, do * 512:(do + 1) * 512],
                        yt[:])

                # Startup: QK for pair 0 only (double-buffered across the
                # qk/rb banks, both idle until attention starts).  V groups
                # are interleaved into pr0/qb0's kj loop below.
                for i, ch in enumerate(range(4)):
                    emit_qk_group(0, ch, wq_t, qt_t[0],
                                  tag=("qk" if i % 2 == 0 else "rb"))
                    emit_qk_group(0, ch, wk_t, kt_t[0],
                                  tag=("rb" if i % 2 == 0 else "qk"))

                # ---------------- attention ----------------
                # Filler PE work (next pair's QK, the output projection) is
                # pumped into the kj loops: the exp on ScalarE (1038ns/tile)
                # outpaces scores+AV on PE (852ns), so PE has idle slots.
                for pr in range(NPAIR):
                    ca65 = (2 * pr) * 65
                    cb65 = (2 * pr + 1) * 65
                    # Fillers are emitted in 2-matmul chunks (~430ns) so they
                    # slot into the per-kj PE idle (~190ns+) without lumping.
                    fillers = []

                    def qk_chunks(prn, ch, w_t, dst):
                        co = ch * 512
                        ps_box = []

                        def chunk(lo, lo2, ps_box=ps_box):
                            if lo == 0:
                                ps_box.append(psp.tile([P, 512], f32,
                                                       tag="qk",
                                                       name="qk_ps"))
                            ps = ps_box[0]
                            for db in (lo, lo2):
                                nc.tensor.matmul(
                                    ps[:],
                                    lhsT=w_t[db][:, prn * P:(prn + 1) * P],
                                    rhs=xt_t[db][:, co:co + 512],
                                    start=(db == 0), stop=(db == DB - 1))
                            if lo2 == DB - 1:
                                nc.vector.tensor_copy(
                                    dst[:, co:co + 512], ps[:])
                        return [lambda lo=lo: chunk(lo, lo + 1)
                                for lo in range(0, DB, 2)]

                    if pr < NPAIR - 1:
                        prn = pr + 1
                        for ch in range(4):
                            fillers += qk_chunks(prn, ch, wq_t, qt_t[prn])
                            fillers += qk_chunks(prn, ch, wk_t, kt_t[prn])
                        if pr == 0:
                            # 32 chunks over qb1..3 (48 steps): 2 of every 3
                            pump_ok = lambda s: s >= 16 and (s - 16) % 3 != 2
                        else:
                            pump_ok = lambda s: s % 2 == 0
                    else:
                        for tv in range(8):
                            for do in range(2):
                                fillers.append(
                                    lambda tv=tv, do=do: emit_proj_piece(
                                        0, tv, do, tag="qk"))
                        # proj half 0 needs ot(*,0), complete after qb1
                        pump_ok = lambda s: s >= 32 and s % 2 == 0
                    fillers.reverse()  # pop() from the front
                    step = 0
                    for qb in range(4):
                        qo = qb * 512
                        out_a = psp.tile([P, 512], f32, tag="outa",
                                         name="out_a")
                        out_b = psp.tile([P, 512], f32, tag="outb",
                                         name="out_b")
                        e_tiles = {}

                        def emit_scores(kj):
                            ko = kj * P
                            s_ab = psp.tile([P, 1024], f32, tag="s", bufs=2,
                                            name="s_ab")
                            nc.tensor.matmul(
                                s_ab[:, 0:512],
                                lhsT=kt_t[pr][0:64, ko:ko + P],
                                rhs=qt_t[pr][0:64, qo:qo + 512],
                                start=True, stop=True)
                            nc.tensor.matmul(
                                s_ab[:, 512:1024],
                                lhsT=kt_t[pr][64:128, ko:ko + P],
                                rhs=qt_t[pr][64:128, qo:qo + 512],
                                start=True, stop=True)
                            if no_exp:
                                e_ab = e_const
                            else:
                                e_ab = att_sb.tile([P, 1024], bf16, tag="e",
                                                   bufs=8, name="e_ab")
                                nc.scalar.activation(e_ab[:], s_ab[:],
                                                     AF.Exp, scale=SCALE)
                            e_tiles[kj] = e_ab

                        def emit_av(kj):
                            e_ab = e_tiles.pop(kj)
                            st = (kj == 0)
                            sp = (kj == 15)
                            nc.tensor.matmul(
                                out_a[0:65, :],
                                lhsT=v_t[kj][:, ca65:ca65 + 65],
                                rhs=e_ab[:, 0:512], start=st, stop=sp)
                            nc.tensor.matmul(
                                out_b[0:65, :],
                                lhsT=v_t[kj][:, cb65:cb65 + 65],
                                rhs=e_ab[:, 512:1024], start=st, stop=sp)

                        if pr == 0 and qb == 0:
                            emit_v_group(0, tag="qk")
                        for kj in range(16):
                            emit_scores(kj)
                            if pr == 0 and qb == 0:
                                # V groups just-in-time for emit_av
                                if kj < 15:
                                    g = kj + 1
                                    emit_v_group(
                                        g, tag="qk" if g % 2 == 0 else "rb")
                            elif fillers and pump_ok(step):
                                fillers.pop()()
                            if kj > 0:
                                emit_av(kj - 1)
                            step += 1
                        emit_av(15)
                        # normalize out^T by 1/denom (PSUM row 64 of each)
                        rec = att_sb.tile([33, 512], f32, tag="rec", bufs=2,
                                          name="rec")
                        nc.vector.reciprocal(rec[0:1, :], out_a[64:65, :])
                        nc.vector.reciprocal(rec[32:33, :], out_b[64:65, :])
                        recb = att_sb.tile([33, 512], bf16, tag="recb",
                                           bufs=2, name="recb")
                        with nc.allow_low_precision(
                                reason="softmax denom recip bf16"):
                            nc.vector.tensor_copy(recb[0:1, :], rec[0:1, :])
                            nc.vector.tensor_copy(recb[32:33, :],
                                                  rec[32:33, :])
                        rb_ps = psp.tile([P, 512], f32, tag="rb",
                                         name="rb_ps")
                        nc.tensor.matmul(rb_ps[0:64, :], lhsT=ones64[0:1, :],
                                         rhs=recb[0:1, :], start=True,
                                         stop=True)
                        nc.tensor.matmul(rb_ps[64:128, :],
                                         lhsT=ones64[32:33, :],
                                         rhs=recb[32:33, :], start=True,
                                         stop=True)
                        rb = att_sb.tile([P, 512], f32, tag="rb", bufs=2,
                                         name="rb")
                        nc.vector.tensor_copy(rb[:], rb_ps[:])
                        half = (qb % 2) * 512
                        ot = ot_t[(pr, qb // 2)]
                        nc.vector.tensor_tensor(
                            ot[0:64, half:half + 512],
                            out_a[0:64, :], rb[0:64, :], ALU.mult)
                        nc.vector.tensor_tensor(
                            ot[64:128, half:half + 512],
                            out_b[0:64, :], rb[64:128, :], ALU.mult)

                # tail: output projection half 1 (ot(*,1) complete after qb3)
                for tv in range(8):
                    for do in range(2):
                        emit_proj_piece(1, tv, do, tag="s")

    nc.compile()
    return nc


def _make_runner(nc):
    import jax
    from jax.sharding import Mesh, NamedSharding, PartitionSpec
    try:
        from jax import shard_map
        _shard_map = lambda f, mesh, in_specs, out_specs: shard_map(
            f, mesh=mesh, in_specs=in_specs, out_specs=out_specs,
            check_vma=False)
    except ImportError:
        from jax.experimental.shard_map import shard_map
        _shard_map = lambda f, mesh, in_specs, out_specs: shard_map(
            f, mesh=mesh, in_specs=in_specs, out_specs=out_specs,
            check_rep=False)
    import concourse.mybir as mybir
    from concourse.bass2jax import (_bass_exec_p, install_neuronx_cc_hook,
                                    partition_id_tensor)

    install_neuronx_cc_hook()

    partition_name = (nc.partition_id_tensor.name
                      if nc.partition_id_tensor else None)
    in_names, out_names, out_avals = [], [], []
    for alloc in nc.m.functions[0].allocations:
        if not isinstance(alloc, mybir.MemoryLocationSet):
            continue
        name = alloc.memorylocations[0].name
        if alloc.kind == "ExternalInput":
            if name != partition_name:
                in_names.append(name)
        elif alloc.kind == "ExternalOutput":
            out_names.append(name)
            out_avals.append(jax.core.ShapedArray(
                tuple(alloc.tensor_shape), mybir.dt.np(alloc.dtype)))

    n_params = len(in_names)
    all_in_names = list(in_names) + list(out_names)
    if partition_name is not None:
        all_in_names.append(partition_name)

    def _body(*args):
        operands = list(args)
        if partition_name is not None:
            operands.append(partition_id_tensor())
        outs = _bass_exec_p.bind(
            *operands,
            out_avals=tuple(out_avals),
            in_names=tuple(all_in_names),
            out_names=tuple(out_names),
            lowering_input_output_aliases=(),
            sim_require_finite=True,
            sim_require_nnan=True,
            nc=nc,
        )
        return tuple(outs)

    devices = jax.devices()[:N_CORES]
    mesh = Mesh(np.asarray(devices), ("core",))
    n_outs = len(out_avals)
    in_specs = (PartitionSpec("core"),) * (n_params + n_outs)
    out_specs = (PartitionSpec("core"),) * n_outs
    sharded = jax.jit(
        _shard_map(_body, mesh, in_specs, out_specs), keep_unused=True)
    sh = NamedSharding(mesh, PartitionSpec("core"))
    return {
        "fn": sharded,
        "in_names": in_names,
        "out_names": out_names,
        "out_avals": out_avals,
        "sharding": sh,
        "mesh": mesh,
    }


def _get_runner(repeat=1):
    key = ("runner", repeat)
    if key not in _CACHE:
        _CACHE[key] = _make_runner(_build_nc(repeat=repeat))
    return _CACHE[key]


def _shard_inputs(x, w_qkv, b_qkv, w_proj, b_proj):
    """Returns concatenated (along axis 0) per-core input arrays, in the
    order of the runner's in_names (xt, wq, wk, wv, wp)."""
    import ml_dtypes
    bf = ml_dtypes.bfloat16

    assert not np.any(np.asarray(b_qkv)), \
        "nonzero b_qkv not supported by this kernel build"

    per = {"xt": [], "wq": [], "wk": [], "wv": [], "wp": []}
    xtb = [np.ascontiguousarray(np.asarray(x)[b].T).astype(bf)
           for b in range(B)]
    w_qkv = np.asarray(w_qkv)
    w_proj = np.asarray(w_proj)
    wslices = {}
    for hh in range(2):
        wslices[("wq", hh)] = np.ascontiguousarray(
            w_qkv[:, 0 * D + hh * 512:0 * D + (hh + 1) * 512]).astype(bf)
        wslices[("wk", hh)] = np.ascontiguousarray(
            w_qkv[:, 1 * D + hh * 512:1 * D + (hh + 1) * 512]).astype(bf)
        wslices[("wv", hh)] = np.ascontiguousarray(
            w_qkv[:, 2 * D + hh * 512:2 * D + (hh + 1) * 512]).astype(bf)
        wslices[("wp", hh)] = np.ascontiguousarray(
            w_proj[hh * 512:(hh + 1) * 512, :]).astype(bf)
    for c in range(N_CORES):
        b, hh = divmod(c, 2)
        per["xt"].append(xtb[b])
        per["wq"].append(wslices[("wq", hh)])
        per["wk"].append(wslices[("wk", hh)])
        per["wv"].append(wslices[("wv", hh)])
        per["wp"].append(wslices[("wp", hh)])
    return {k: np.concatenate(v, axis=0) for k, v in per.items()}


def _run(runner, shards):
    import jax
    concat_in = [shards[name] for name in runner["in_names"]]
    concat_zeros = [
        np.zeros((N_CORES * av.shape[0],) + tuple(av.shape[1:]), av.dtype)
        for av in runner["out_avals"]
    ]
    outs = runner["fn"](*concat_in, *concat_zeros)
    jax.block_until_ready(outs)
    return {
        name: np.asarray(outs[i]).reshape(
            (N_CORES,) + tuple(runner["out_avals"][i].shape))
        for i, name in enumerate(runner["out_names"])
    }


def kernel(x, w_qkv, b_qkv, w_proj, b_proj):
    x = np.asarray(x)
    runner = _get_runner()
    shards = _shard_inputs(x, w_qkv, b_qkv, w_proj, b_proj)
    outs = _run(runner, shards)
    y = outs["y"]  # [8, S, D] fp32
    full = np.empty((B, S, D), np.float32)
    bp = np.asarray(b_proj, np.float32)
    for b in range(B):
        full[b] = y[2 * b] + y[2 * b + 1] + bp
    return full

